# revision 6
# baseline (speedup 1.0000x reference)
"""Trainium2 Bass kernel for nn_CommNetActor — v2.

Same algebra as v1 (tail folded into per-agent readout matrices Wz; sigmoid
rewritten as tanh with the affine folded into fc1), plus:

- Full-bf16 trunk (input, weights, activations; f32 PSUM accumulate):
  the BIR verifier forbids mixing 32-bit and non-32-bit matmul inputs,
  so bf16 anywhere means bf16 everywhere in the trunk. Enables FWL on
  weight loads, halves activation SBUF and input upload; measured rel
  err 1.6e-4 end-to-end.

- Readout runs Wz-stationary: lgT[c,s] = sum_a Wz_a^T H3_chunk, so the
  expensive per-chunk 128-col weight loads of H3 disappear (Wz slices are
  16-col loads). The per-class bias is folded into the Exp activation's
  bias operand (eb constant eliminated). PE transposes put exp(logits)
  back into [sample, class] orientation for a free-dim softmax.
- 7 DMAs total (2 const packs, 4 input, 1 output) instead of 26; the
  output is staged in SBUF [128, 1024] and stored contiguously once,
  host-side de-interleave replaces the scattered 64B-segment stores.
- Elementwise work in 512-col chunks through 6 single-bank PSUM buffers
  (6 chunk-pipelines in flight), split across ACT (tanh, exp, half of
  fc2/fc3) and DVE (the rest + softmax tail). GPSIMD cannot read PSUM
  on TRN2 hardware (the BIR verifier rejects it), so it gets no
  PSUM->SBUF bias+relu work despite being idle.

Column layout per 1024-sample super-tile unchanged: sample s of agent a
sits at column (s//512)*2048 + a*512 + s%512; input packed two samples
per column ([128, 2048] per super-tile).
"""

import numpy as np

import concourse.bass as bass
import concourse.mybir as mybir
import concourse.tile as tile
from concourse import bacc
from concourse.bass import ts
from concourse.bass_utils import run_bass_kernel_spmd
from concourse.masks import make_identity

B = 65536
A = 4
OBS = 64
D = 128
C = 16
NCORES = 8
BLOC = B // NCORES
ST = 1024
NST = BLOC // ST
COLS = A * ST
NCHUNK = 512
GROUPS = ST // D            # 8 sample-groups of 128 per super-tile
STPERDMA = 1                # super-tiles per input DMA

F32 = mybir.dt.float32
F32R = mybir.dt.float32r
AFT = mybir.ActivationFunctionType
ALU = mybir.AluOpType

TRUNK_DT = mybir.dt.bfloat16
HDT = mybir.dt.bfloat16

# wpack columns: enc(0:128) w1(128:256) w2(256:384) w3(384:512) wz(512:576)
WCOLS = 576

_compiled = {}


def _build_bass():
    nc = bacc.Bacc()

    ot_d = nc.dram_tensor("ot", [2 * OBS, NST * COLS // 2], TRUNK_DT, kind="ExternalInput")
    wp_d = nc.dram_tensor("wpack", [D, WCOLS], TRUNK_DT, kind="ExternalInput")
    bp_d = nc.dram_tensor("bpack", [D, 8], F32, kind="ExternalInput")
    out_d = nc.dram_tensor("probs", [D, NST * GROUPS * C], F32, kind="ExternalOutput")

    with tile.TileContext(nc) as tc:
        with (
            tc.tile_pool(name="consts", bufs=1) as cpool,
            tc.tile_pool(name="ot", bufs=2) as opool,
            tc.tile_pool(name="acts", bufs=2) as hpool,
            tc.tile_pool(name="soft", bufs=2) as spool,
            tc.tile_pool(name="stage", bufs=1) as stpool,
            tc.tile_pool(name="mm", bufs=6, space="PSUM") as mmpool,
            tc.tile_pool(name="lgT", bufs=1, space="PSUM") as lgpool,
            tc.tile_pool(name="tr", bufs=1, space="PSUM") as trpool,
        ):
            wp_t = cpool.tile([D, WCOLS], TRUNK_DT, name="wp")
            nc.sync.dma_start(wp_t[:], wp_d[:])
            bp_t = cpool.tile([D, 8], F32, name="bp")
            nc.sync.dma_start(bp_t[:], bp_d[:])
            ident = cpool.tile([D, D], F32, name="ident")
            make_identity(nc, ident[:])

            ew = wp_t[:, 0:128]
            w1 = wp_t[:, 128:256]
            w2 = wp_t[:, 256:384]
            w3 = wp_t[:, 384:512]
            wz = wp_t[:, 512:576]
            b0 = bp_t[:, 0:1]
            b1 = bp_t[:, 1:2]
            b2 = bp_t[:, 2:3]
            b3 = bp_t[:, 3:4]
            bsm = bp_t[0:C, 4:5]   # softmax bias (folded dec/cl4 bias), rows 0..15

            stage = stpool.tile([D, NST * GROUPS * C], F32, name="stage")

            for dm in range(NST // STPERDMA):
                ot_t = opool.tile([2 * OBS, STPERDMA * COLS // 2], TRUNK_DT, tag="ot")
                nc.sync.dma_start(
                    ot_t[:], ot_d[:, ts(dm, STPERDMA * COLS // 2)],
                )
                for sst in range(STPERDMA):
                    st = dm * STPERDMA + sst
                    otv = ot_t[:, sst * (COLS // 2):(sst + 1) * (COLS // 2)]

                    def ew_op(engine, dst_ap, ps, b, func):
                        if engine == "A":
                            nc.scalar.activation(dst_ap, ps[:], func, bias=b)
                        elif engine == "G":
                            nc.gpsimd.tensor_scalar(
                                dst_ap, ps[:], b, 0.0, ALU.add, ALU.max,
                            )
                        else:
                            nc.vector.tensor_scalar(
                                dst_ap, ps[:], b, 0.0, ALU.add, ALU.max,
                            )

                    # ---- enc: tanh(0.5 x + 0.5 b); K=64 row-group pairs ----
                    # 512-col chunks; alternate row groups so consecutive
                    # matmuls execute concurrently in the PE array.
                    h0 = hpool.tile([D, COLS], HDT, tag="h0")
                    for cb in range(4):
                        for hh in range(2):
                            ps = mmpool.tile([D, NCHUNK], F32, tag="mm")
                            nc.tensor.matmul(
                                ps[:],
                                ew[64 * hh: 64 * (hh + 1), :],
                                otv[64 * hh: 64 * (hh + 1), ts(cb, NCHUNK)],
                                start=True, stop=True,
                            )
                            nc.scalar.activation(
                                h0[:, hh * 2048 + cb * NCHUNK:
                                   hh * 2048 + (cb + 1) * NCHUNK],
                                ps[:], AFT.Tanh, bias=b0, scale=0.5,
                            )

                    # ---- fc trunk; 512-col chunks; EW split ACT/DVE/Pool ----
                    def fc(dst, src, w, b, engines):
                        for j in range(8):
                            ps = mmpool.tile([D, NCHUNK], F32, tag="mm")
                            nc.tensor.matmul(
                                ps[:], w, src[:, ts(j, NCHUNK)],
                                start=True, stop=True,
                            )
                            ew_op(engines[j], dst[:, ts(j, NCHUNK)], ps, b,
                                  AFT.Relu)

                    h1 = hpool.tile([D, COLS], HDT, tag="h1")
                    fc(h1, h0, w1, b1, "VVVVVVVV")
                    h2 = hpool.tile([D, COLS], HDT, tag="h2")
                    fc(h2, h1, w2, b2, "AAVVAAVV")
                    h3 = hpool.tile([D, COLS], HDT, tag="h3")
                    fc(h3, h2, w3, b3, "AAVVAAVV")

                    # ---- readout: Wz-stationary, class-major logits ----
                    for h in range(2):
                        lgT = lgpool.tile([C, NCHUNK], F32, tag="lgT")
                        for a in range(A):
                            nc.tensor.matmul(
                                lgT[:],
                                wz[:, ts(a, C)],
                                h3[:, h * 2048 + a * NCHUNK:
                                   h * 2048 + (a + 1) * NCHUNK],
                                start=(a == 0), stop=(a == A - 1),
                            )
                        # exp(logits + bias) straight out of PSUM
                        e = spool.tile([C, NCHUNK], F32, tag="e")
                        nc.scalar.activation(e[:], lgT[:], AFT.Exp, bias=bsm)
                        # back to [sample, class] via PE transpose
                        tr = trpool.tile([D, 4 * C], F32, tag="tr")
                        for gg in range(4):
                            nc.tensor.transpose(
                                tr[:, ts(gg, C)], e[:, ts(gg, D)], ident[0:C, 0:C],
                            )
                        # softmax tail in free dim
                        s4 = spool.tile([D, 4], F32, tag="s4")
                        nc.vector.reduce_sum(
                            s4[:], tr[:].rearrange("p (g c) -> p g c", c=C),
                            axis=mybir.AxisListType.X,
                        )
                        r4 = spool.tile([D, 4], F32, tag="r4")
                        nc.vector.reciprocal(r4[:], s4[:])
                        nc.vector.tensor_mul(
                            stage[:, st * 128 + h * 64: st * 128 + (h + 1) * 64]
                            .rearrange("p (g c) -> p g c", c=C),
                            tr[:].rearrange("p (g c) -> p g c", c=C),
                            r4[:].unsqueeze(2).broadcast_to([D, 4, C]),
                        )

                    # store the first half early so it overlaps the
                    # second half's compute
                    if st == NST // 2 - 1:
                        half = NST * GROUPS * C // 2
                        nc.sync.dma_start(out_d[:, :half], stage[:, :half])

            half = NST * GROUPS * C // 2
            nc.sync.dma_start(out_d[:, half:], stage[:, half:])

    nc.compile()
    return nc


def _prep_inputs(inputs):
    """Host-side: fused weights + per-core transposed input shards."""
    f64 = lambda x: np.asarray(x, np.float64)
    enc_w, enc_b = f64(inputs["enc_w"]), f64(inputs["enc_b"])
    fc1_w, fc1_b = f64(inputs["fc1_w"]), f64(inputs["fc1_b"])
    fc2_w, fc2_b = f64(inputs["fc2_w"]), f64(inputs["fc2_b"])
    fc3_w, fc3_b = f64(inputs["fc3_w"]), f64(inputs["fc3_b"])
    cl4_w, cl4_b = f64(inputs["cl4_w"]), f64(inputs["cl4_b"])
    dec_w, dec_b = f64(inputs["dec_w"]), f64(inputs["dec_b"])

    A_ = cl4_w[:D]
    Bm = cl4_w[D:]
    Da = dec_w.reshape(A, D, C)
    Dsum = Da.sum(0)
    Wz = np.concatenate(
        [A_ @ Da[a] + 0.25 * (Bm @ (Dsum - Da[a])) for a in range(A)], axis=1
    )  # [128, 64]
    bias_p = dec_b + cl4_b @ Dsum  # [16]

    import ml_dtypes
    wpack = np.concatenate(
        [np.vstack([enc_w, enc_w]), 0.5 * fc1_w, fc2_w, fc3_w, Wz], axis=1
    ).astype(ml_dtypes.bfloat16)
    bpack = np.zeros((D, 8), np.float32)
    bpack[:, 0] = 0.5 * enc_b
    bpack[:, 1] = fc1_b + 0.5 * fc1_w.sum(0)
    bpack[:, 2] = fc2_b
    bpack[:, 3] = fc3_b
    bpack[:C, 4] = bias_p

    O = np.asarray(inputs["O"], np.float32)  # [B, A, OBS]
    in_maps = []
    for c in range(NCORES):
        oc = O[c * BLOC: (c + 1) * BLOC]
        x = oc.reshape(NST, 2, ST // 2, A, OBS)
        ot = np.ascontiguousarray(
            x.transpose(1, 4, 0, 3, 2).astype(ml_dtypes.bfloat16)
        ).reshape(2 * OBS, NST * COLS // 2)
        in_maps.append({"ot": ot, "wpack": wpack, "bpack": bpack})
    return in_maps


def _unstage(arr):
    """[128, NST*GROUPS*16] staging -> [BLOC, 16].

    staging col = st*128 + h*64 + gg*16 + c holds sample
    st*1024 + h*512 + gg*128 + p."""
    x = arr.reshape(D, NST, 2, 4, C)
    return np.ascontiguousarray(x.transpose(1, 2, 3, 0, 4)).reshape(BLOC, C)


def kernel(**inputs):
    if "nc" not in _compiled:
        _compiled["nc"] = _build_bass()
    nc = _compiled["nc"]
    in_maps = _prep_inputs(inputs)
    res = run_bass_kernel_spmd(nc, in_maps, core_ids=list(range(NCORES)))
    return np.concatenate(
        [_unstage(res.results[i]["probs"]) for i in range(NCORES)], axis=0
    )
